# revision 6
# baseline (speedup 1.0000x reference)
"""HashedLinear TRN2 kernel: out = x @ w[indx] + b on 8 NeuronCores.

Sharding: units (output) dim across 8 cores. Each core: x^T replicated,
w replicated (as a per-partition SBUF table), its 512-unit slice of indx/b.

Device algorithm per core:
  1. ap_gather (GPSIMD, d=8): gather bf16 octs w[8*(k>>3) .. +8] for every
     element of the core's indx slice. Each Q7 core's index list covers 2
     W-rows per instruction (J=1024); output is 16x-replicated per block.
  2. DMA compaction: move the 8 useful partition-rows per instruction into
     W-candidate k-tiles [128 rows, 512 units x 8 cands] (bf16).
  3. DVE select tree (3 levels, host-shipped bf16 bit-plane masks) picks the
     right candidate -> W k-tile [128, 512] bf16.
  4. PE matmul: out[b,u] accumulated over 32 k-tiles into 8 PSUM banks
     (lhsT = x^T tile cast to bf16, rhs = W k-tile).
  5. Bias add + DMA out.
"""

import numpy as np
import ml_dtypes

BATCH, IN_DIM, UNITS, NW = 1024, 4096, 4096, 65536
NCORES = 8
UPC = UNITS // NCORES          # 512 units per core
D = 8                          # gather octs
NE = NW // D                   # 8192 table entries of 8 bf16
J = 1024                       # gather indices per Q7-core list per instruction
ROWS_PER_INST = 16             # W rows covered per ap_gather instruction
T_INST = IN_DIM // ROWS_PER_INST   # 256 gather instructions
INST_PER_KTILE = 128 // ROWS_PER_INST  # 8
KTILES = IN_DIM // 128         # 32
MTILES = BATCH // 128          # 8

_cached = {}


def _build():
    import concourse.bacc as bacc
    import concourse.mybir as mybir
    import concourse.tile as tile

    nc = bacc.Bacc("TRN2", target_bir_lowering=False, debug=False,
                   num_devices=NCORES)
    dt = mybir.dt
    with tile.TileContext(nc) as tc:
        xT_d = nc.dram_tensor("xT", [IN_DIM, BATCH], dt.float32, kind="ExternalInput")
        wtb_d = nc.dram_tensor("wtb", [128, NW], dt.bfloat16, kind="ExternalInput")
        idx_d = nc.dram_tensor("idxq", [128, T_INST * (J // 16)], dt.int16, kind="ExternalInput")
        m2_d = nc.dram_tensor("m2", [IN_DIM, UPC], dt.uint8, kind="ExternalInput")
        m1_d = nc.dram_tensor("m1", [IN_DIM, UPC], dt.uint8, kind="ExternalInput")
        m0_d = nc.dram_tensor("m0", [IN_DIM, UPC], dt.uint8, kind="ExternalInput")
        b_d = nc.dram_tensor("brep", [128, UPC], dt.float32, kind="ExternalInput")
        out_d = nc.dram_tensor("out", [BATCH, UPC], dt.float32, kind="ExternalOutput")

        with (
            tc.tile_pool(name="tblp", bufs=1) as tblp,
            tc.tile_pool(name="idxp", bufs=2) as idxp,
            tc.tile_pool(name="gp", bufs=1) as gp,
            tc.tile_pool(name="cp", bufs=1) as cp,
            tc.tile_pool(name="selp", bufs=1) as selp,
            tc.tile_pool(name="xp", bufs=2) as xp,
            tc.tile_pool(name="mp", bufs=2) as mp,
            tc.tile_pool(name="bp", bufs=1) as bp,
            tc.tile_pool(name="op", bufs=2) as op,
            tc.tile_pool(name="ps", bufs=1, space="PSUM") as ps,
        ):
            tbl = tblp.tile([128, NW], dt.bfloat16, tag="tbl")
            h = NW // 2
            nc.sync.dma_start(tbl[:, :h], wtb_d.ap()[:, :h])
            nc.sync.dma_start(tbl[:, h:], wtb_d.ap()[:, h:])
            bias = bp.tile([128, UPC], dt.float32, tag="bias")
            nc.sync.dma_start(bias[:, :], b_d.ap()[:, :])

            psum = []
            for m in range(MTILES):
                pt = ps.tile([128, UPC], dt.float32, tag=f"ps{m}", name=f"psum{m}")
                psum.append(pt)

            for t2 in range(KTILES):
                # --- gather + compact this k-tile's candidates ---
                C = cp.tile([128, UPC * D], dt.bfloat16, tag="C")
                ichunk = idxp.tile([128, INST_PER_KTILE * (J // 16)], dt.int16, tag="ichunk")
                c0 = t2 * INST_PER_KTILE * (J // 16)
                nc.sync.dma_start(ichunk[:, :], idx_d.ap()[:, c0:c0 + INST_PER_KTILE * (J // 16)])
                for ti in range(INST_PER_KTILE):
                    t = t2 * INST_PER_KTILE + ti
                    G = gp.tile([128, J * D], dt.bfloat16, tag="G")
                    nc.gpsimd.ap_gather(
                        out_ap=G[:, :].rearrange("p (j e) -> p j e", e=D),
                        in_ap=tbl[:, :].rearrange("p (n e) -> p n e", e=D),
                        idxs_ap=ichunk[:, ti * (J // 16):(ti + 1) * (J // 16)],
                        channels=128, num_elems=NE, d=D, num_idxs=J,
                    )
                    r0 = ti * ROWS_PER_INST
                    nc.sync.dma_start(
                        C[r0:r0 + ROWS_PER_INST, :],
                        G[0:128:16, :],
                    )
                # --- select tree ---
                k0 = t2 * 128
                m2t = mp.tile([128, UPC], dt.uint8, tag="m2")
                m1t = mp.tile([128, UPC], dt.uint8, tag="m1")
                m0t = mp.tile([128, UPC], dt.uint8, tag="m0")
                nc.sync.dma_start(m2t[:, :], m2_d.ap()[k0:k0 + 128, :])
                nc.sync.dma_start(m1t[:, :], m1_d.ap()[k0:k0 + 128, :])
                nc.sync.dma_start(m0t[:, :], m0_d.ap()[k0:k0 + 128, :])
                c3 = C[:, :].rearrange("p (u e) -> p u e", e=D)
                s4 = selp.tile([128, UPC * 4], dt.bfloat16, tag="s4")
                s4v = s4[:, :].rearrange("p (u e) -> p u e", e=4)
                nc.vector.select(
                    s4v, m2t[:, :].unsqueeze(-1).broadcast_to((128, UPC, 4)),
                    c3[:, :, 4:8], c3[:, :, 0:4])
                s2 = selp.tile([128, UPC * 2], dt.bfloat16, tag="s2")
                s2v = s2[:, :].rearrange("p (u e) -> p u e", e=2)
                nc.vector.select(
                    s2v, m1t[:, :].unsqueeze(-1).broadcast_to((128, UPC, 2)),
                    s4v[:, :, 2:4], s4v[:, :, 0:2])
                Wt = selp.tile([128, UPC], dt.bfloat16, tag="Wt")
                nc.vector.select(
                    Wt[:, :], m0t[:, :],
                    s2v[:, :, 1], s2v[:, :, 0])
                # --- x^T tile stream + cast ---
                xf = xp.tile([128, BATCH], dt.float32, tag="xf")
                nc.sync.dma_start(xf[:, :], xT_d.ap()[k0:k0 + 128, :])
                xb = xp.tile([128, BATCH], dt.bfloat16, tag="xb")
                nc.vector.tensor_copy(xb[:, :], xf[:, :])
                # --- matmuls ---
                for m in range(MTILES):
                    nc.tensor.matmul(
                        psum[m][:, :], xb[:, m * 128:(m + 1) * 128], Wt[:, :],
                        start=(t2 == 0), stop=(t2 == KTILES - 1))

            for m in range(MTILES):
                ot = op.tile([128, UPC], dt.float32, tag="ot")
                nc.vector.tensor_add(ot[:, :], psum[m][:, :], bias[:, :])
                nc.sync.dma_start(out_d.ap()[m * 128:(m + 1) * 128, :], ot[:, :])
    nc.compile()
    return nc


def _prep_inputs(x, w, b, indx):
    xT = np.ascontiguousarray(x.T).astype(np.float32, copy=False)
    w_oct = w.astype(ml_dtypes.bfloat16)          # table values (bf16 cast)
    wtb = np.broadcast_to(w_oct, (128, NW)).copy()
    in_maps = []
    for c in range(NCORES):
        sub = indx[:, c * UPC:(c + 1) * UPC].astype(np.int64)
        idxq = (sub >> 3).astype(np.int16)        # oct index
        m2 = ((sub >> 2) & 1).astype(np.uint8)
        m1 = ((sub >> 1) & 1).astype(np.uint8)
        m0 = (sub & 1).astype(np.uint8)
        # wrapped gather-list layout: [T_INST, 8 cores, 2 rows, 512] ->
        # list_j = rows-major; wrapped[16c2+p, t*64+s] = list[t,c2,s*16+p]
        A = idxq.reshape(T_INST, 8, 2, UPC).reshape(T_INST, 8, J)
        wrapped = np.transpose(A.reshape(T_INST, 8, J // 16, 16), (1, 3, 0, 2))
        wrapped = np.ascontiguousarray(wrapped).reshape(128, T_INST * (J // 16))
        brep = np.broadcast_to(b[c * UPC:(c + 1) * UPC].astype(np.float32),
                               (128, UPC)).copy()
        in_maps.append({
            "xT": xT, "wtb": wtb, "idxq": wrapped,
            "m2": np.ascontiguousarray(m2), "m1": np.ascontiguousarray(m1),
            "m0": np.ascontiguousarray(m0), "brep": brep,
        })
    return in_maps


def kernel(x, w, b, indx):
    from concourse import bass_utils
    if "nc" not in _cached:
        _cached["nc"] = _build()
    in_maps = _prep_inputs(x, w, b, indx)
    res = bass_utils.run_bass_kernel_spmd(
        _cached["nc"], in_maps, core_ids=list(range(NCORES)))
    out = np.concatenate([res.results[c]["out"] for c in range(NCORES)], axis=1)
    return out.astype(np.float32)


# revision 7
# speedup vs baseline: 1.0177x; 1.0177x over previous
"""HashedLinear TRN2 kernel: out = x @ w[indx] + b on 8 NeuronCores.

Sharding: units (output) dim across 8 cores. Each core: x^T replicated,
w replicated (as a per-partition SBUF table), its 512-unit slice of indx/b.

Device algorithm per core:
  1. ap_gather (GPSIMD, d=8): gather bf16 octs w[8*(k>>3) .. +8] for every
     element of the core's indx slice. Each Q7 core's index list covers 2
     W-rows per instruction (J=1024); output is 16x-replicated per block.
  2. DMA compaction: move the 8 useful partition-rows per instruction into
     W-candidate k-tiles [128 rows, 512 units x 8 cands] (bf16).
  3. DVE select tree (3 levels, host-shipped bf16 bit-plane masks) picks the
     right candidate -> W k-tile [128, 512] bf16.
  4. PE matmul: out[b,u] accumulated over 32 k-tiles into 8 PSUM banks
     (lhsT = x^T tile cast to bf16, rhs = W k-tile).
  5. Bias add + DMA out.
"""

import numpy as np
import ml_dtypes

BATCH, IN_DIM, UNITS, NW = 1024, 4096, 4096, 65536
NCORES = 8
UPC = UNITS // NCORES          # 512 units per core
D = 2                          # gather pairs
NE = NW // D                   # 32768 table entries of 2 bf16
J = 2048                       # gather indices per Q7-core list per instruction
ROWS_PER_INST = 32             # W rows covered per ap_gather instruction
T_INST = IN_DIM // ROWS_PER_INST   # 256 gather instructions
INST_PER_KTILE = 128 // ROWS_PER_INST  # 8
KTILES = IN_DIM // 128         # 32
MTILES = BATCH // 128          # 8

_cached = {}


def _build():
    import concourse.bacc as bacc
    import concourse.mybir as mybir
    import concourse.tile as tile

    nc = bacc.Bacc("TRN2", target_bir_lowering=False, debug=False,
                   num_devices=NCORES)
    dt = mybir.dt
    with tile.TileContext(nc) as tc:
        xT_d = nc.dram_tensor("xT", [IN_DIM, BATCH], dt.float32, kind="ExternalInput")
        wtb_d = nc.dram_tensor("wtb", [128, NW], dt.bfloat16, kind="ExternalInput")
        idx_d = nc.dram_tensor("idxq", [128, T_INST * (J // 16)], dt.int16, kind="ExternalInput")
        m0_d = nc.dram_tensor("m0", [IN_DIM, UPC], dt.uint8, kind="ExternalInput")
        b_d = nc.dram_tensor("brep", [128, UPC], dt.float32, kind="ExternalInput")
        out_d = nc.dram_tensor("out", [BATCH, UPC], dt.float32, kind="ExternalOutput")

        with (
            tc.tile_pool(name="tblp", bufs=1) as tblp,
            tc.tile_pool(name="idxp", bufs=2) as idxp,
            tc.tile_pool(name="gp", bufs=2) as gp,
            tc.tile_pool(name="cp", bufs=2) as cp,
            tc.tile_pool(name="selp", bufs=1) as selp,
            tc.tile_pool(name="xp", bufs=2) as xp,
            tc.tile_pool(name="mp", bufs=2) as mp,
            tc.tile_pool(name="bp", bufs=1) as bp,
            tc.tile_pool(name="op", bufs=2) as op,
            tc.tile_pool(name="ps", bufs=1, space="PSUM") as ps,
        ):
            tbl = tblp.tile([128, NW], dt.bfloat16, tag="tbl")
            h = NW // 2
            nc.sync.dma_start(tbl[:, :h], wtb_d.ap()[:, :h])
            nc.sync.dma_start(tbl[:, h:], wtb_d.ap()[:, h:])
            bias = bp.tile([128, UPC], dt.float32, tag="bias")
            nc.sync.dma_start(bias[:, :], b_d.ap()[:, :])

            psum = []
            for m in range(MTILES):
                pt = ps.tile([128, UPC], dt.float32, tag=f"ps{m}", name=f"psum{m}")
                psum.append(pt)

            for t2 in range(KTILES):
                # --- gather + compact this k-tile's candidates ---
                C = cp.tile([128, UPC * D], dt.bfloat16, tag="C")
                ichunk = idxp.tile([128, INST_PER_KTILE * (J // 16)], dt.int16, tag="ichunk")
                c0 = t2 * INST_PER_KTILE * (J // 16)
                nc.sync.dma_start(ichunk[:, :], idx_d.ap()[:, c0:c0 + INST_PER_KTILE * (J // 16)])
                for ti in range(INST_PER_KTILE):
                    t = t2 * INST_PER_KTILE + ti
                    G = gp.tile([128, J * D], dt.bfloat16, tag="G")
                    nc.gpsimd.ap_gather(
                        out_ap=G[:, :].rearrange("p (j e) -> p j e", e=D),
                        in_ap=tbl[:, :].rearrange("p (n e) -> p n e", e=D),
                        idxs_ap=ichunk[:, ti * (J // 16):(ti + 1) * (J // 16)],
                        channels=128, num_elems=NE, d=D, num_idxs=J,
                    )
                    r0 = ti * ROWS_PER_INST
                    nc.sync.dma_start(
                        C[r0:r0 + ROWS_PER_INST, :],
                        G[0:128:16, :],
                    )
                # --- select tree ---
                k0 = t2 * 128
                m0t = mp.tile([128, UPC], dt.uint8, tag="m0")
                nc.sync.dma_start(m0t[:, :], m0_d.ap()[k0:k0 + 128, :])
                c3 = C[:, :].rearrange("p (u e) -> p u e", e=D)
                Wt = selp.tile([128, UPC], dt.bfloat16, tag="Wt")
                nc.vector.select(
                    Wt[:, :], m0t[:, :],
                    c3[:, :, 1], c3[:, :, 0])
                # --- x^T tile stream + cast ---
                xf = xp.tile([128, BATCH], dt.float32, tag="xf")
                nc.sync.dma_start(xf[:, :], xT_d.ap()[k0:k0 + 128, :])
                xb = xp.tile([128, BATCH], dt.bfloat16, tag="xb")
                nc.vector.tensor_copy(xb[:, :], xf[:, :])
                # --- matmuls ---
                for m in range(MTILES):
                    nc.tensor.matmul(
                        psum[m][:, :], xb[:, m * 128:(m + 1) * 128], Wt[:, :],
                        start=(t2 == 0), stop=(t2 == KTILES - 1))

            for m in range(MTILES):
                ot = op.tile([128, UPC], dt.float32, tag="ot")
                nc.vector.tensor_add(ot[:, :], psum[m][:, :], bias[:, :])
                nc.sync.dma_start(out_d.ap()[m * 128:(m + 1) * 128, :], ot[:, :])
    nc.compile()
    return nc


def _prep_inputs(x, w, b, indx):
    xT = np.ascontiguousarray(x.T).astype(np.float32, copy=False)
    w_oct = w.astype(ml_dtypes.bfloat16)          # table values (bf16 cast)
    wtb = np.broadcast_to(w_oct, (128, NW)).copy()
    in_maps = []
    for c in range(NCORES):
        sub = indx[:, c * UPC:(c + 1) * UPC].astype(np.int64)
        idxq = (sub >> 1).astype(np.int16)        # pair index
        m0 = (sub & 1).astype(np.uint8)
        # wrapped gather-list layout: [T_INST, 8 cores, 2 rows, 512] ->
        # list_j = rows-major; wrapped[16c2+p, t*64+s] = list[t,c2,s*16+p]
        A = idxq.reshape(T_INST, 8, J // UPC, UPC).reshape(T_INST, 8, J)
        wrapped = np.transpose(A.reshape(T_INST, 8, J // 16, 16), (1, 3, 0, 2))
        wrapped = np.ascontiguousarray(wrapped).reshape(128, T_INST * (J // 16))
        brep = np.broadcast_to(b[c * UPC:(c + 1) * UPC].astype(np.float32),
                               (128, UPC)).copy()
        in_maps.append({
            "xT": xT, "wtb": wtb, "idxq": wrapped,
            "m0": np.ascontiguousarray(m0), "brep": brep,
        })
    return in_maps


def kernel(x, w, b, indx):
    from concourse import bass_utils
    if "nc" not in _cached:
        _cached["nc"] = _build()
    in_maps = _prep_inputs(x, w, b, indx)
    res = bass_utils.run_bass_kernel_spmd(
        _cached["nc"], in_maps, core_ids=list(range(NCORES)))
    out = np.concatenate([res.results[c]["out"] for c in range(NCORES)], axis=1)
    return out.astype(np.float32)


# revision 8
# speedup vs baseline: 1.2650x; 1.2430x over previous
"""HashedLinear TRN2 kernel: out = x @ w[indx] + b on 8 NeuronCores.

Sharding: units (output) dim across 8 cores. Each core: x^T replicated,
w replicated (as a per-partition SBUF table), its 512-unit slice of indx/b.

Device algorithm per core:
  1. ap_gather (GPSIMD, d=2): for every element of the core's indx slice,
     gather the bf16 pair w[2*(k>>1) .. +2] from a per-partition 128KiB
     table (ap_gather indices are int16, so the raw 16-bit index is shifted
     to pair granularity). Each Q7 core's list covers 4 W-rows per
     instruction (J=2048); output is 16x-replicated within each block.
  2. DMA compaction: move the 8 useful partition-rows per instruction into
     W-candidate k-tiles [128 rows, 512 units x 2 cands] (bf16).
  3. One DVE select (uint8 low-bit mask, shipped from host) picks the right
     pair half -> W k-tile [128, 512] bf16.
  4. PE matmul: out[b,u] accumulated over 32 k-tiles into 8 PSUM banks
     (lhsT = x^T tile cast to bf16, rhs = W k-tile).
  5. Bias add + DMA out.

The gather is the bottleneck: cayman's Q7 SBUF read path serializes
RD_CMDs (~102 cyc per 4 indices, measured ~26 ns/idx), so 2M gathers/core
cost ~6.8 ms regardless of batching; DMA, selects, and matmul all hide
under it. Larger gather payloads (d=4/8) were measured slower per index
(62 ns/idx at d=8) and add select levels, so d=2 is the optimum.
"""

import numpy as np
import ml_dtypes

BATCH, IN_DIM, UNITS, NW = 1024, 4096, 4096, 65536
NCORES = 8
UPC = UNITS // NCORES          # 512 units per core
D = 2                          # gather pairs
NE = NW // D                   # 32768 table entries of 2 bf16
J = 2048                       # gather indices per Q7-core list per instruction
ROWS_PER_INST = 32             # W rows covered per ap_gather instruction
T_INST = IN_DIM // ROWS_PER_INST   # 256 gather instructions
INST_PER_KTILE = 128 // ROWS_PER_INST  # 8
KTILES = IN_DIM // 128         # 32
MTILES = BATCH // 128          # 8

_cached = {}


def _build():
    import concourse.bacc as bacc
    import concourse.mybir as mybir
    import concourse.tile as tile

    nc = bacc.Bacc("TRN2", target_bir_lowering=False, debug=False,
                   num_devices=NCORES)
    dt = mybir.dt
    with tile.TileContext(nc) as tc:
        xT_d = nc.dram_tensor("xT", [IN_DIM, BATCH], dt.float32, kind="ExternalInput")
        wtb_d = nc.dram_tensor("wtb", [128, NW], dt.bfloat16, kind="ExternalInput")
        idx_d = nc.dram_tensor("idxq", [128, T_INST * (J // 16)], dt.int16, kind="ExternalInput")
        m0_d = nc.dram_tensor("m0", [IN_DIM, UPC], dt.uint8, kind="ExternalInput")
        b_d = nc.dram_tensor("brep", [128, UPC], dt.float32, kind="ExternalInput")
        out_d = nc.dram_tensor("out", [BATCH, UPC], dt.float32, kind="ExternalOutput")

        with (
            tc.tile_pool(name="tblp", bufs=1) as tblp,
            tc.tile_pool(name="idxp", bufs=2) as idxp,
            tc.tile_pool(name="gp", bufs=2) as gp,
            tc.tile_pool(name="cp", bufs=2) as cp,
            tc.tile_pool(name="selp", bufs=1) as selp,
            tc.tile_pool(name="xp", bufs=2) as xp,
            tc.tile_pool(name="mp", bufs=2) as mp,
            tc.tile_pool(name="bp", bufs=1) as bp,
            tc.tile_pool(name="op", bufs=2) as op,
            tc.tile_pool(name="ps", bufs=1, space="PSUM") as ps,
        ):
            tbl = tblp.tile([128, NW], dt.bfloat16, tag="tbl")
            h = NW // 2
            nc.sync.dma_start(tbl[:, :h], wtb_d.ap()[:, :h])
            nc.sync.dma_start(tbl[:, h:], wtb_d.ap()[:, h:])
            bias = bp.tile([128, UPC], dt.float32, tag="bias")
            nc.sync.dma_start(bias[:, :], b_d.ap()[:, :])

            psum = []
            for m in range(MTILES):
                pt = ps.tile([128, UPC], dt.float32, tag=f"ps{m}", name=f"psum{m}")
                psum.append(pt)

            for t2 in range(KTILES):
                # --- gather + compact this k-tile's candidates ---
                C = cp.tile([128, UPC * D], dt.bfloat16, tag="C")
                ichunk = idxp.tile([128, INST_PER_KTILE * (J // 16)], dt.int16, tag="ichunk")
                c0 = t2 * INST_PER_KTILE * (J // 16)
                nc.sync.dma_start(ichunk[:, :], idx_d.ap()[:, c0:c0 + INST_PER_KTILE * (J // 16)])
                for ti in range(INST_PER_KTILE):
                    t = t2 * INST_PER_KTILE + ti
                    G = gp.tile([128, J * D], dt.bfloat16, tag="G")
                    nc.gpsimd.ap_gather(
                        out_ap=G[:, :].rearrange("p (j e) -> p j e", e=D),
                        in_ap=tbl[:, :].rearrange("p (n e) -> p n e", e=D),
                        idxs_ap=ichunk[:, ti * (J // 16):(ti + 1) * (J // 16)],
                        channels=128, num_elems=NE, d=D, num_idxs=J,
                    )
                    r0 = ti * ROWS_PER_INST
                    nc.sync.dma_start(
                        C[r0:r0 + ROWS_PER_INST, :],
                        G[0:128:16, :],
                    )
                # --- select tree ---
                k0 = t2 * 128
                m0t = mp.tile([128, UPC], dt.uint8, tag="m0")
                nc.sync.dma_start(m0t[:, :], m0_d.ap()[k0:k0 + 128, :])
                c3 = C[:, :].rearrange("p (u e) -> p u e", e=D)
                Wt = selp.tile([128, UPC], dt.bfloat16, tag="Wt")
                nc.vector.select(
                    Wt[:, :], m0t[:, :],
                    c3[:, :, 1], c3[:, :, 0])
                # --- x^T tile stream + cast ---
                xf = xp.tile([128, BATCH], dt.float32, tag="xf")
                nc.sync.dma_start(xf[:, :], xT_d.ap()[k0:k0 + 128, :])
                xb = xp.tile([128, BATCH], dt.bfloat16, tag="xb")
                nc.vector.tensor_copy(xb[:, :], xf[:, :])
                # --- matmuls ---
                for m in range(MTILES):
                    nc.tensor.matmul(
                        psum[m][:, :], xb[:, m * 128:(m + 1) * 128], Wt[:, :],
                        start=(t2 == 0), stop=(t2 == KTILES - 1))

            for m in range(MTILES):
                ot = op.tile([128, UPC], dt.float32, tag="ot")
                nc.vector.tensor_add(ot[:, :], psum[m][:, :], bias[:, :])
                nc.sync.dma_start(out_d.ap()[m * 128:(m + 1) * 128, :], ot[:, :])
    nc.compile()
    return nc


def _prep_inputs(x, w, b, indx):
    xT = np.ascontiguousarray(x.T).astype(np.float32, copy=False)
    w_oct = w.astype(ml_dtypes.bfloat16)          # table values (bf16 cast)
    wtb = np.broadcast_to(w_oct, (128, NW)).copy()
    in_maps = []
    for c in range(NCORES):
        sub = indx[:, c * UPC:(c + 1) * UPC].astype(np.int64)
        idxq = (sub >> 1).astype(np.int16)        # pair index
        m0 = (sub & 1).astype(np.uint8)
        # wrapped gather-list layout: [T_INST, 8 cores, 2 rows, 512] ->
        # list_j = rows-major; wrapped[16c2+p, t*64+s] = list[t,c2,s*16+p]
        A = idxq.reshape(T_INST, 8, J // UPC, UPC).reshape(T_INST, 8, J)
        wrapped = np.transpose(A.reshape(T_INST, 8, J // 16, 16), (1, 3, 0, 2))
        wrapped = np.ascontiguousarray(wrapped).reshape(128, T_INST * (J // 16))
        brep = np.broadcast_to(b[c * UPC:(c + 1) * UPC].astype(np.float32),
                               (128, UPC)).copy()
        in_maps.append({
            "xT": xT, "wtb": wtb, "idxq": wrapped,
            "m0": np.ascontiguousarray(m0), "brep": brep,
        })
    return in_maps


def kernel(x, w, b, indx):
    from concourse import bass_utils
    if "nc" not in _cached:
        _cached["nc"] = _build()
    in_maps = _prep_inputs(x, w, b, indx)
    res = bass_utils.run_bass_kernel_spmd(
        _cached["nc"], in_maps, core_ids=list(range(NCORES)))
    out = np.concatenate([res.results[c]["out"] for c in range(NCORES)], axis=1)
    return out.astype(np.float32)
